# revision 3
# baseline (speedup 1.0000x reference)
"""Trainium2 Bass kernel v2 for BertSelfAttentionSubstitute (relu^2 attention).

Full inputs in, full output out. 8 cores: core i handles batch b=i//2,
heads 8*(i%2)..8*(i%2)+7 (4 head PAIRS).

Key structure vs v1:
- QT/KT kept in SBUF as bf16 [128, 2048] per head-pair (head A rows 0:64,
  head B rows 64:128). bf16 operands are essential: packed row-group MM
  pairs share the streaming XBUS, and two bf16 streams fit where two
  f32r streams serialize (measured 75 ns/MM packed vs 305 ns/MM plain).
- Scores: row-group packed MM pairs (contraction=64): head A on array rows
  0:63, head B on rows 64:127, running concurrently; both write one
  [128, 1024] 2-bank PSUM tile.
- probs = relu(s)^2 via fused scalar_tensor_tensor (s max 0)*s on DVE, or
  ACT relu + Pool square, rotating per PROB_PATTERN to balance engines.
- ctx: col-group packed MM pairs (M=64): head A -> psum partitions 0:64,
  head B -> 64:128 of the same bank, accumulated over j, staged to SBUF
  (ACT/DVE copies; DMA cannot read PSUM) then DMA'd out.
- Software pipelining: ctx MMs for k-tile j are emitted after scores MMs
  for k-tile j+1 so the PE never waits on the elementwise engines.
"""

import sys
import numpy as np

sys.path.insert(0, "/opt/trn_rl_repo")

N_CORES = 8
B, S, D_MODEL = 4, 2048, 1024
NH_LOCAL, HD, DOUT = 8, 64, 512
P = 128
DIN_CHUNKS = D_MODEL // P  # 8
DOUT_TILES = DOUT // P  # 4 (also = number of head pairs)
TOKC = 512
NTOKC = S // TOKC  # 4
NK = S // P  # 16 k-tiles
NPAIR = 4

# probs engine rotation per score pair-tile (relu must read PSUM -> ACT/DVE
# only; Pool is SBUF-only):
#   A = ACT relu + Pool square
#   B = DVE relu + Pool square
#   C = ACT relu + DVE square
PROB_PATTERN = "ABCBACBABCABCBAC"  # 5 A, 6 B, 5 C per 16

_CACHE = {}


def _emit(nc, tc, mybir, xt, xtb, wqt, wkt, wvt, out, loop_n=None, seed=None):
    f32 = mybir.dt.float32
    f32r = mybir.dt.float32r
    bf16 = mybir.dt.bfloat16
    Relu = mybir.ActivationFunctionType.Relu
    Max = mybir.AluOpType.max
    Mult = mybir.AluOpType.mult

    with tc.tile_pool(name="persist", bufs=1) as persist, \
         tc.tile_pool(name="xtp", bufs=2) as xtp, \
         tc.tile_pool(name="elem", bufs=4) as elem:

        if seed is not None:
            # timing mode: fill internal DRAM inputs from the small seed
            sx = persist.tile([P, TOKC], f32, tag="seedx", name="seedx")
            sw = persist.tile([P, TOKC], f32, tag="seedw", name="seedw")
            nc.sync.dma_start(sx[:], seed[:, 0:TOKC])
            nc.sync.dma_start(sw[:], seed[:, TOKC:2 * TOKC])
            sxb = persist.tile([P, TOKC], bf16, tag="seedxb", name="seedxb")
            nc.vector.tensor_copy(sxb[:], sx[:])
            swb = persist.tile([P, TOKC], bf16, tag="seedwb", name="seedwb")
            nc.vector.tensor_copy(swb[:], sw[:])
            for d in range(DIN_CHUNKS):
                for c in range(NTOKC):
                    nc.sync.dma_start(
                        xtb[d * P:(d + 1) * P, c * TOKC:(c + 1) * TOKC],
                        sxb[:])
                for wap in (wqt, wkt, wvt):
                    nc.sync.dma_start(wap[d * P:(d + 1) * P, :], swb[:])

        def body():
            # --- persistent tiles ---
            w_tiles = {}
            for wname, wap, wdt in (("q", wqt, bf16), ("k", wkt, bf16),
                                    ("v", wvt, bf16)):
                for d in range(DIN_CHUNKS):
                    t = persist.tile([P, DOUT], wdt, tag=f"w{wname}{d}",
                                     name=f"w{wname}{d}")
                    nc.scalar.dma_start(t[:], wap[d * P:(d + 1) * P, :])
                    w_tiles[(wname, d)] = t

            qt_sb = [persist.tile([P, S], bf16, tag=f"qt{t}", name=f"qt{t}")
                     for t in range(NPAIR)]
            kt_sb = [persist.tile([P, S], bf16, tag=f"kt{t}", name=f"kt{t}")
                     for t in range(NPAIR)]
            v_sb = [persist.tile([P, DOUT], bf16, tag=f"v{t}", name=f"v{t}")
                    for t in range(NK)]

            # --- Stage B: projections ---
            cp = 0  # ACT/DVE copy rotation counter

            def psum_copy(dst_ap, src_ap):
                nonlocal cp
                if cp % 2 == 0:
                    nc.vector.tensor_copy(dst_ap, src_ap)
                else:
                    nc.scalar.copy(dst_ap, src_ap)
                cp += 1

            with tc.tile_pool(name="psA", bufs=2, space="PSUM") as psA:
                for c in range(NTOKC):
                    xbc = []
                    for d in range(DIN_CHUNKS):
                        tb = xtp.tile([P, TOKC], bf16, tag=f"xb{d}",
                                      name=f"xb{d}")
                        nc.sync.dma_start(
                            tb[:], xtb[d * P:(d + 1) * P, c * TOKC:(c + 1) * TOKC])
                        xbc.append(tb)
                    for wname, dst in (("q", qt_sb), ("k", kt_sb)):
                        for tt in range(DOUT_TILES):
                            ps = psA.tile([P, TOKC], f32, tag="proj", name="ps")
                            for d in range(DIN_CHUNKS):
                                nc.tensor.matmul(
                                    ps[:],
                                    lhsT=w_tiles[(wname, d)][:, tt * P:(tt + 1) * P],
                                    rhs=xbc[d][:],
                                    start=(d == 0), stop=(d == DIN_CHUNKS - 1))
                            psum_copy(dst[tt][:, c * TOKC:(c + 1) * TOKC],
                                      ps[:])
                    for tt in range(TOKC // P):
                        ps = psA.tile([P, DOUT], f32, tag="projv", name="psv")
                        for d in range(DIN_CHUNKS):
                            nc.tensor.matmul(
                                ps[:],
                                lhsT=xbc[d][:, tt * P:(tt + 1) * P],
                                rhs=w_tiles[("v", d)][:],
                                start=(d == 0), stop=(d == DIN_CHUNKS - 1))
                        psum_copy(v_sb[c * (TOKC // P) + tt][:], ps[:])

            # --- Stage C: attention, head pairs, software pipelined ---
            m = 0
            with tc.tile_pool(name="psS", bufs=2, space="PSUM") as psS, \
                 tc.tile_pool(name="psC", bufs=1, space="PSUM") as psC:
                for t in range(NPAIR):
                    ctx = [psC.tile([P, TOKC], f32, tag=f"ctx{c}",
                                    name=f"ctx{c}") for c in range(NTOKC)]
                    kt_t = kt_sb[t]
                    qt_t = qt_sb[t]

                    def emit_scores(j, c):
                        nonlocal m
                        pp = psS.tile([P, 2 * TOKC], f32, tag="pp", name="pp")
                        q0 = c * TOKC
                        nc.tensor.matmul(
                            pp[:, 0:TOKC],
                            lhsT=kt_t[0:HD, j * P:(j + 1) * P],
                            rhs=qt_t[0:HD, q0:q0 + TOKC],
                            start=True, stop=True)
                        nc.tensor.matmul(
                            pp[:, TOKC:2 * TOKC],
                            lhsT=kt_t[HD:P, j * P:(j + 1) * P],
                            rhs=qt_t[HD:P, q0:q0 + TOKC],
                            start=True, stop=True)
                        pr = elem.tile([P, 2 * TOKC], bf16, tag="pr", bufs=8,
                                       name="pr")
                        rl = elem.tile([P, 2 * TOKC], bf16, tag="rl",
                                       bufs=3, name="rl")
                        kind = PROB_PATTERN[m % len(PROB_PATTERN)]
                        m += 1
                        if kind == "A":
                            nc.scalar.activation(rl[:], pp[:], Relu)
                            nc.gpsimd.tensor_mul(pr[:], rl[:], rl[:])
                        elif kind == "B":
                            nc.vector.tensor_scalar_max(rl[:], pp[:], 0.0)
                            nc.gpsimd.tensor_mul(pr[:], rl[:], rl[:])
                        else:
                            nc.scalar.activation(rl[:], pp[:], Relu)
                            nc.vector.tensor_mul(pr[:], rl[:], rl[:])
                        return pr

                    def emit_ctx(j, c, pr):
                        nc.tensor.matmul(
                            ctx[c][0:HD, :],
                            lhsT=v_sb[j][:, t * P:t * P + HD],
                            rhs=pr[:, 0:TOKC],
                            start=(j == 0), stop=(j == NK - 1),
                            skip_group_check=True)
                        nc.tensor.matmul(
                            ctx[c][HD:P, :],
                            lhsT=v_sb[j][:, t * P + HD:(t + 1) * P],
                            rhs=pr[:, TOKC:2 * TOKC],
                            start=(j == 0), stop=(j == NK - 1),
                            skip_group_check=True)

                    prev = None  # probs of k-tile j-1: list of 4 pr tiles
                    for j in range(NK):
                        cur = []
                        for half in range(2):
                            for c in (2 * half, 2 * half + 1):
                                cur.append(emit_scores(j, c))
                            if prev is not None:
                                for c in (2 * half, 2 * half + 1):
                                    emit_ctx(j - 1, c, prev[c])
                        prev = cur
                    for c in range(NTOKC):
                        emit_ctx(NK - 1, c, prev[c])

                    # drain ctx: PSUM -> SBUF (ACT/DVE) -> DRAM
                    ostage = elem.tile([P, S], f32, tag="ostage", bufs=1,
                                       name="ostage")
                    for c in range(NTOKC):
                        psum_copy(ostage[:, c * TOKC:(c + 1) * TOKC],
                                  ctx[c][:])
                    dq = (nc.scalar, nc.gpsimd, nc.sync, nc.scalar)[t]
                    dq.dma_start(out[t * P:(t + 1) * P, :], ostage[:])

        if loop_n is not None:
            with tc.For_i(0, loop_n, 1):
                body()
        else:
            body()


def _build(loop_n=None, internal_io=False):
    key = ("nc", loop_n, internal_io)
    if key in _CACHE:
        return _CACHE[key]
    import concourse.tile as tile
    from concourse import bacc, mybir

    f32 = mybir.dt.float32
    f32r = mybir.dt.float32r
    bf16 = mybir.dt.bfloat16

    nc = bacc.Bacc("TRN2", target_bir_lowering=False, debug=False,
                   num_devices=N_CORES)
    ikind = "Internal" if internal_io else "ExternalInput"
    xt = None
    xtb = nc.dram_tensor("xtb", [D_MODEL, S], bf16, kind=ikind).ap()
    wqt = nc.dram_tensor("wqt", [D_MODEL, DOUT], bf16, kind=ikind).ap()
    wkt = nc.dram_tensor("wkt", [D_MODEL, DOUT], bf16, kind=ikind).ap()
    wvt = nc.dram_tensor("wvt", [D_MODEL, DOUT], bf16, kind=ikind).ap()
    out = nc.dram_tensor("out", [DOUT, S], f32, kind="ExternalOutput").ap()
    seed = None
    if internal_io:
        seed = nc.dram_tensor("seed", [P, 2 * TOKC], f32,
                              kind="ExternalInput").ap()

    with tile.TileContext(nc) as tc:
        _emit(nc, tc, mybir, xt, xtb, wqt, wkt, wvt, out, loop_n=loop_n,
              seed=seed)

    nc.compile()
    _CACHE[key] = nc
    return nc


def _in_maps(hidden_states, Wq, Wk, Wv):
    import ml_dtypes
    maps = []
    for i in range(N_CORES):
        b = i // 2
        rows = slice(DOUT * (i % 2), DOUT * (i % 2) + DOUT)
        xt = np.ascontiguousarray(hidden_states[b].T)
        maps.append({
            "xtb": xt.astype(ml_dtypes.bfloat16),
            "wqt": (np.ascontiguousarray(Wq[rows].T) / 8.0).astype(
                ml_dtypes.bfloat16),
            "wkt": np.ascontiguousarray(Wk[rows].T).astype(ml_dtypes.bfloat16),
            "wvt": np.ascontiguousarray(Wv[rows].T).astype(ml_dtypes.bfloat16),
        })
    return maps


def kernel(hidden_states, attention_mask, Wq, bq, Wk, bk, Wv, bv):
    # attention_mask / biases are structurally zero for this problem spec.
    from concourse.bass_utils import run_bass_kernel_spmd

    nc = _build()
    hidden_states = np.asarray(hidden_states, dtype=np.float32)
    maps = _in_maps(hidden_states,
                    np.asarray(Wq, np.float32),
                    np.asarray(Wk, np.float32),
                    np.asarray(Wv, np.float32))
    res = run_bass_kernel_spmd(nc, maps, core_ids=list(range(N_CORES)))
    out = np.empty((B, S, D_MODEL), np.float32)
    for i in range(N_CORES):
        b = i // 2
        cols = slice(DOUT * (i % 2), DOUT * (i % 2) + DOUT)
        out[b, :, cols] = res.results[i]["out"].T
    return out
